# revision 16
# baseline (speedup 1.0000x reference)
"""Bass/Trainium2 kernel for HCFC-GNN (3-layer GCN + hierarchy max-constraint).

v2 design (8 NeuronCores, SPMD), restructured around measured platform costs:
  - Tables live in SBUF node-major ([128 tok, 392 ranks x 256B]); per-edge
    gathers use SBUF-source dma_gather (transpose mode) spread over 4 SWDGE
    queues (~2x faster than one queue; ~6x faster than HBM-source gathers).
  - Gather output is feature-major msg_T[f, e]; instead of transposing, the
    NEXT layer's weight transform is applied per edge tile:
    mw[e, f2] = matmul(lhsT=msg_T_tile, rhs=W) - msg_T is exactly the lhsT.
  - Scatter-add via one-hot S built from colrel (DVE is_equal), accumulated
    transposed: acc_T[f2, t] += matmul(lhsT=mw_bf16, rhs=S).
  - Per-edge bias b*dinv_src is exact via one extra matmul per block:
    acc_T += b_l (x) colsum[t], colsum[t] = sum_{e->t} dinv[src] (host-built).
  - relu commutes with the positive dinv_t scale, so the next table is just
    T_next = dinv^2 (.) relu(acc_T) (DVE mult with a host-fed replicated
    dinv^2 tile) - no per-block transposes or separate transform phase.
  - Layer-1 table (dinv (.) x, zero-padded to 128 feats) is fully
    host-precomputed: no AllGather for layer 1.
  - Layers 2,3: table chunks written feature-major to DRAM, AllGather, then
    ONE dma-transpose load per layer rebuilds the node-major SBUF table.
"""

import os
import numpy as np
import ml_dtypes

N = 50000
E = 1600000
C = 13
DIN = 12
H = 128
NCORES = 8
SH = N // NCORES          # 6250 nodes per shard
CH = 6272                 # shard chunk rows (6250 + 22 zero pad), 49*128
BLK = CH // 128           # 49 blocks per shard
LASTB = SH - (BLK - 1) * 128  # 106 valid rows in the last block
NR = NCORES * BLK         # 392 table ranks
HALF = 4 * CH             # 25088 tokens per gather half (int16-safe)
ZROW = SH                 # local zero-token index inside each half
PADCREL = 300.0           # colrel value guaranteed not to match iota 0..127
GMAX = 14                 # max tiles per gather call (SWDGE ring safety)

bf16 = ml_dtypes.bfloat16

LAST_RESULTS = None


def _prep_edges(edge_index):
    """Partition/sort edges; build per-core gather-index and colrel streams with
    block/half slot sizes (TL) uniform across cores so one SPMD program works."""
    row = np.concatenate([edge_index[0], np.arange(N, dtype=np.int32)])
    col = np.concatenate([edge_index[1], np.arange(N, dtype=np.int32)])
    deg = np.bincount(row, minlength=N).astype(np.float32)

    s_shard = row // SH
    grow = s_shard * CH + (row % SH)       # token index in table [0, 8*CH)
    half = (grow >= HALF).astype(np.int64)
    gloc = np.where(half == 0, grow, grow - HALF).astype(np.int64)
    tcore = col // SH
    tcol = col % SH
    blk = tcol // 128
    crel = (tcol % 128).astype(np.int64)

    key = ((tcore * BLK) + blk) * 2 + half
    order = np.lexsort((gloc, key))
    key_s = key[order]
    gloc_s = gloc[order]
    crel_s = crel[order]

    nslots = NCORES * BLK * 2
    cnt = np.bincount(key_s, minlength=nslots).reshape(NCORES, BLK, 2)
    starts = np.zeros(nslots + 1, np.int64)
    np.cumsum(cnt.reshape(-1), out=starts[1:])

    # uniform tile counts across cores
    TL = np.maximum(1, ((cnt + 127) // 128).max(axis=0))  # [BLK, 2]
    off = np.zeros((BLK, 2), np.int64)                    # slot offsets in tiles
    tot = [0, 0]
    for h in (0, 1):
        for b in range(BLK):
            off[b, h] = tot[h]
            tot[h] += TL[b, h]


    gidx = []   # per core: ((gidx_lo, crel_lo), (gidx_hi, crel_hi))
    for k in range(NCORES):
        per_half = []
        for h in (0, 1):
            gparts, cparts = [], []
            for b in range(BLK):
                s = starts[(k * BLK + b) * 2 + h]
                e = starts[(k * BLK + b) * 2 + h + 1]
                n = int(e - s)
                m = int(TL[b, h]) * 128
                # pad tokens are killed by S=0 (crel=PADCREL never matches
                # iota), so any index works; spread them over all partitions
                # and ranks to avoid same-address descriptor contention
                gseg = (np.arange(m, dtype=np.int64) * 97 + b * 1009) % HALF
                cseg = np.full(m, PADCREL, np.float64)
                gseg[:n] = gloc_s[s:e]
                cseg[:n] = crel_s[s:e]
                # wrapped idx layout: pos i -> partition i%16, col i//16
                gparts.append(gseg.reshape(m // 16, 16).T.astype(np.int16))
                # colrel layout: pos i -> partition i%128, col i//128
                cparts.append(cseg.reshape(m // 128, 128).T.astype(bf16))
            g = np.hstack(gparts)                      # [16, tot_h*8]
            per_half.append((np.tile(g, (8, 1)).copy(), np.hstack(cparts).copy()))
        gidx.append(per_half)
    return deg, row, col, TL, off, tot, gidx


def _build_program(TL, off):
    import concourse.bacc as bacc
    import concourse.mybir as mybir
    import concourse.tile as tile

    dt = mybir.dt
    nc = bacc.Bacc("TRN2", target_bir_lowering=False, debug=False,
                   num_devices=NCORES, num_swdge_queues=4)

    TOT = [int(off[-1, 0] + TL[-1, 0]), int(off[-1, 1] + TL[-1, 1])]

    # ---- inputs ----
    T1in = nc.dram_tensor("T1in", [128, NR * 128], dt.bfloat16, kind="ExternalInput")
    W1Tp = nc.dram_tensor("W1Tp", [128, H], dt.bfloat16, kind="ExternalInput")
    W2T = nc.dram_tensor("W2T", [H, H], dt.bfloat16, kind="ExternalInput")
    W3Tp = nc.dram_tensor("W3Tp", [H, 16], dt.bfloat16, kind="ExternalInput")
    b1r = nc.dram_tensor("b1r", [1, H], dt.bfloat16, kind="ExternalInput")
    b2r = nc.dram_tensor("b2r", [1, H], dt.bfloat16, kind="ExternalInput")
    b3r = nc.dram_tensor("b3r", [1, 16], dt.bfloat16, kind="ExternalInput")
    dinv2b = nc.dram_tensor("dinv2b", [128, BLK * 128], dt.bfloat16,
                            kind="ExternalInput")
    colsum = nc.dram_tensor("colsum", [1, BLK * 128], dt.bfloat16,
                            kind="ExternalInput")
    iota_in = nc.dram_tensor("iota_in", [128, 128], dt.bfloat16, kind="ExternalInput")
    identf = nc.dram_tensor("identf", [16, 16], dt.float32, kind="ExternalInput")
    Rfl = nc.dram_tensor("Rfl", [128, C * C], dt.float32, kind="ExternalInput")
    gi_lo = nc.dram_tensor("gi_lo", [128, 8 * TOT[0]], dt.int16,
                           kind="ExternalInput")
    gi_hi = nc.dram_tensor("gi_hi", [128, 8 * TOT[1]], dt.int16,
                           kind="ExternalInput")
    cr_lo = nc.dram_tensor("cr_lo", [128, TOT[0]], dt.bfloat16,
                           kind="ExternalInput")
    cr_hi = nc.dram_tensor("cr_hi", [128, TOT[1]], dt.bfloat16,
                           kind="ExternalInput")
    out = nc.dram_tensor("out", [SH, C], dt.float32, kind="ExternalOutput")

    # per-rank feature-major table chunks for the collective
    ginT = nc.dram_tensor("ginT", [CH, 128], dt.bfloat16)
    goutT = nc.dram_tensor("goutT", [NCORES * CH, 128], dt.bfloat16,
                           addr_space="Shared")

    qn = [0]

    def next_q():
        qn[0] = (qn[0] + 1) % 4
        return qn[0]

    with tile.TileContext(nc) as tc:
        with (
            tc.tile_pool(name="const", bufs=1) as cpool,
            tc.tile_pool(name="idx", bufs=1) as ipool,
            tc.tile_pool(name="tbl", bufs=1) as tblpool,
            tc.tile_pool(name="mT", bufs=8) as mpool,
            tc.tile_pool(name="sbl", bufs=2) as spool,
            tc.tile_pool(name="mwb", bufs=4) as wbpool,
            tc.tile_pool(name="gblk", bufs=2) as gpool,
            tc.tile_pool(name="fin", bufs=1) as fpool,
            tc.tile_pool(name="mwp", bufs=2, space="PSUM") as wpsum,
            tc.tile_pool(name="accp", bufs=4, space="PSUM") as apool,
            tc.tile_pool(name="tp3", bufs=2, space="PSUM") as tpsum,
        ):
            # ---- constants ----
            w1_t = cpool.tile([128, H], dt.bfloat16)
            nc.sync.dma_start(out=w1_t[:], in_=W1Tp[:])
            w2_t = cpool.tile([H, H], dt.bfloat16)
            nc.sync.dma_start(out=w2_t[:], in_=W2T[:])
            w3_t = cpool.tile([H, 16], dt.bfloat16)
            nc.sync.dma_start(out=w3_t[:], in_=W3Tp[:])
            b1_t = cpool.tile([1, H], dt.bfloat16)
            nc.sync.dma_start(out=b1_t[:], in_=b1r[:])
            b2_t = cpool.tile([1, H], dt.bfloat16)
            nc.sync.dma_start(out=b2_t[:], in_=b2r[:])
            b3_t = cpool.tile([1, 16], dt.bfloat16)
            nc.sync.dma_start(out=b3_t[:], in_=b3r[:])
            d2_t = cpool.tile([128, BLK * 128], dt.bfloat16)
            nc.sync.dma_start(out=d2_t[:], in_=dinv2b[:])
            cs_t = cpool.tile([1, BLK * 128], dt.bfloat16)
            nc.sync.dma_start(out=cs_t[:], in_=colsum[:])
            io_t = cpool.tile([128, 128], dt.bfloat16)
            nc.sync.dma_start(out=io_t[:], in_=iota_in[:])
            idf_t = cpool.tile([16, 16], dt.float32)
            nc.sync.dma_start(out=idf_t[:], in_=identf[:])
            r_t = cpool.tile([128, C * C], dt.float32)
            nc.sync.dma_start(out=r_t[:], in_=Rfl[:])
            gil_t = ipool.tile([128, 8 * TOT[0]], dt.int16)
            nc.sync.dma_start(out=gil_t[:], in_=gi_lo[:])
            gih_t = ipool.tile([128, 8 * TOT[1]], dt.int16)
            nc.sync.dma_start(out=gih_t[:], in_=gi_hi[:])
            crl_t = ipool.tile([128, TOT[0]], dt.bfloat16)
            nc.sync.dma_start(out=crl_t[:], in_=cr_lo[:])
            crh_t = ipool.tile([128, TOT[1]], dt.bfloat16)
            nc.sync.dma_start(out=crh_t[:], in_=cr_hi[:])

            halves = ((gil_t, crl_t, 0), (gih_t, crh_t, 196))

            def agg_block(tbl_t, b, wt, bt, width, par):
                """Aggregate target block b with per-edge weight transform.
                Returns PSUM acc_T [width, 128] (features x targets)."""
                acc = apool.tile([128, 128], dt.float32, tag="acc")
                first = True
                for hh in (0, 1):
                    gi_t, cr_t, rk0 = halves[hh]
                    tl = int(TL[b, hh])
                    o = int(off[b, hh])
                    nch = -(-tl // GMAX)
                    for ci in range(nch):
                        c0 = tl * ci // nch
                        cn = tl * (ci + 1) // nch - c0
                        oc = o + c0
                        mT = mpool.tile([128, 1, GMAX * 128], dt.bfloat16,
                                        tag="mT")
                        nc.gpsimd.dma_gather(
                            out_ap=mT[:, :, 0:cn * 128],
                            in_ap=tbl_t[:, rk0:rk0 + 196, :],
                            idxs_ap=gi_t[:, oc * 8:(oc + cn) * 8],
                            num_idxs=cn * 128, num_idxs_reg=cn * 128,
                            elem_size=H, transpose=True,
                            sbuf_tokens_per_rank=128,
                            sbuf_free_dim_per_rank=256,
                            single_packet=False, queue_num=next_q(),
                        )
                        S = spool.tile([128, GMAX, 128], dt.bfloat16, tag="sb")
                        nc.vector.tensor_tensor(
                            out=S[:, 0:cn, :],
                            in0=cr_t[:, oc:oc + cn].unsqueeze(2)
                                .broadcast_to([128, cn, 128]),
                            in1=io_t[:, :].unsqueeze(1)
                                .broadcast_to([128, cn, 128]),
                            op=mybir.AluOpType.is_equal,
                        )
                        for j0 in range(0, cn, 4):
                            jn = min(4, cn - j0)
                            mw = wpsum.tile([128, 4, H], dt.float32, tag="mw")
                            for i in range(jn):
                                nc.tensor.matmul(
                                    mw[:, i, 0:width],
                                    mT[:, 0, (j0 + i) * 128:(j0 + i + 1) * 128],
                                    wt[:, 0:width], start=True, stop=True)
                            mwb = wbpool.tile([128, 4, H], dt.bfloat16,
                                              tag="mwb")
                            if par[0] % 2 == 0:
                                nc.vector.tensor_copy(mwb[:, 0:jn, 0:width],
                                                      mw[:, 0:jn, 0:width])
                            else:
                                nc.scalar.activation(
                                    mwb[:, 0:jn, 0:width], mw[:, 0:jn, 0:width],
                                    mybir.ActivationFunctionType.Copy)
                            par[0] += 1
                            for i in range(jn):
                                nc.tensor.matmul(
                                    acc[0:width, :], mwb[:, i, 0:width],
                                    S[:, j0 + i, :], start=first, stop=False)
                                first = False
                # exact bias: acc_T += b (x) colsum[t]
                nc.tensor.matmul(acc[0:width, :], bt[:, 0:width],
                                 cs_t[:, b * 128:(b + 1) * 128],
                                 start=False, stop=True)
                return acc

            def load_table_from_collective():
                nc.gpsimd.collective_compute(
                    "AllGather", mybir.AluOpType.bypass,
                    replica_groups=[list(range(NCORES))],
                    ins=[ginT[:, :]], outs=[goutT[:, :]],
                )
                t = tblpool.tile([128, NR, 128], dt.bfloat16, tag="tbl")
                nc.sync.dma_start(out=t[:, :, :], in_=goutT[:, :],
                                  transpose=True)
                return t

            # ---------------- layer 1: table comes straight from the host
            tbl1 = tblpool.tile([128, NR, 128], dt.bfloat16, tag="tbl")
            nc.sync.dma_start(out=tbl1[:, :, :],
                              in_=T1in[:, :].rearrange("p (a b) -> p a b",
                                                       b=128))

            GB = 4
            par = [0]
            for lyr, (tbl_ref, wt, bt) in enumerate(
                    ((tbl1, w1_t, b1_t), (None, w2_t, b2_t))):
                tbl_t = tbl_ref if tbl_ref is not None \
                    else load_table_from_collective()
                for g0 in range(0, BLK, GB):
                    gn = min(GB, BLK - g0)
                    stage = gpool.tile([128, GB, 128], dt.bfloat16, tag="u")
                    for bi in range(gn):
                        acc = agg_block(tbl_t, g0 + bi, wt, bt, H, par)
                        nc.scalar.activation(
                            stage[:, bi, :], acc[:, :],
                            mybir.ActivationFunctionType.Relu)
                    g = gpool.tile([128, GB, 128], dt.bfloat16, tag="g")
                    nc.vector.tensor_tensor(
                        out=g[:, 0:gn, :], in0=stage[:, 0:gn, :],
                        in1=d2_t[:, g0 * 128:(g0 + gn) * 128]
                            .rearrange("p (a b) -> p a b", b=128),
                        op=mybir.AluOpType.mult)
                    nc.sync.dma_start(
                        out=ginT[g0 * 128:(g0 + gn) * 128, :]
                            .rearrange("(a f) t -> f a t", f=128),
                        in_=g[:, 0:gn, :])

            # ---------------- layer 3 + sigmoid + hierarchy max
            tbl3 = load_table_from_collective()
            for g0 in range(0, BLK, GB):
                gn = min(GB, BLK - g0)
                t4 = tpsum.tile([128, GB * 16], dt.float32, tag="t4")
                for bi in range(gn):
                    b = g0 + bi
                    acc = agg_block(tbl3, b, w3_t, b3_t, 16, par)
                    d1b = fpool.tile([16, 128], dt.float32, tag="d1b")
                    nc.scalar.sqrt(d1b[:, :],
                                   d2_t[0:16, b * 128:(b + 1) * 128])
                    s3 = fpool.tile([16, 128], dt.float32, tag="s3")
                    nc.vector.tensor_tensor(
                        out=s3[:, :], in0=acc[0:16, :],
                        in1=d1b[:, :],
                        op=mybir.AluOpType.mult)
                    nc.tensor.transpose(t4[:, bi * 16:(bi + 1) * 16],
                                        s3[:, :], idf_t[:, :])
                h34 = fpool.tile([128, GB * 16], dt.float32, tag="h34")
                nc.scalar.activation(h34[:, 0:gn * 16], t4[:, 0:gn * 16],
                                     mybir.ActivationFunctionType.Sigmoid)
                tmp = fpool.tile([128, GB, C, C], dt.float32, tag="tmp")
                nc.vector.tensor_tensor(
                    out=tmp[:, 0:gn, :, :],
                    in0=h34[:, :].rearrange("p (a c) -> p a c", c=16)
                        [:, 0:gn, 0:C].unsqueeze(2)
                        .broadcast_to([128, gn, C, C]),
                    in1=r_t[:, :].rearrange("p (a b) -> p a b", a=C)
                        .unsqueeze(1).broadcast_to([128, gn, C, C]),
                    op=mybir.AluOpType.mult,
                )
                o4 = fpool.tile([128, GB, C], dt.float32, tag="o4")
                nc.vector.tensor_reduce(o4[:, 0:gn, :], tmp[:, 0:gn, :, :],
                                        axis=mybir.AxisListType.X,
                                        op=mybir.AluOpType.max)
                rows = min(SH, (g0 + gn) * 128) - g0 * 128
                nfull = rows // 128
                if nfull > 0:
                    nc.sync.dma_start(
                        out=out[g0 * 128:g0 * 128 + nfull * 128, :]
                            .rearrange("(a t) c -> t a c", t=128),
                        in_=o4[:, 0:nfull, :])
                rem = rows - nfull * 128
                if rem > 0:
                    nc.sync.dma_start(
                        out=out[g0 * 128 + nfull * 128:g0 * 128 + rows, :],
                        in_=o4[0:rem, nfull, :])

    nc.compile()
    return nc


def kernel(x, edge_index, R, W1, b1, W2, b2, W3, b3, **_):
    global LAST_RESULTS
    import concourse.mybir  # noqa: F401  (ensure env importable early)
    from concourse.bass_utils import run_bass_kernel_spmd

    x = np.asarray(x, np.float32)
    edge_index = np.asarray(edge_index, np.int32)
    deg, row, col, TL, off, tot, gidx = _prep_edges(edge_index)

    nc = _build_program(TL, off)

    dinv = (1.0 / np.sqrt(deg)).astype(np.float64)
    colsum_full = np.bincount(col, weights=dinv[row], minlength=N)

    # layer-1 table: dinv*x, node-major swizzled [128 tok, 392 ranks x 128 f]
    xw = (dinv[:, None] * x).astype(np.float32)            # [N, DIN]
    xp = np.zeros([NCORES, CH, 128], np.float32)
    xp[:, :SH, :DIN] = xw.reshape(NCORES, SH, DIN)
    T1 = np.ascontiguousarray(
        xp.reshape(NCORES, BLK, 128, 128).transpose(2, 0, 1, 3)
        .reshape(128, NR * 128)).astype(bf16)

    W1p = np.zeros([128, H], np.float32)
    W1p[:DIN, :] = np.asarray(W1, np.float32).T
    W1p = W1p.astype(bf16)
    W2Tb = np.ascontiguousarray(np.asarray(W2, np.float32).T).astype(bf16)
    W3p = np.zeros([H, 16], np.float32)
    W3p[:, :C] = np.asarray(W3, np.float32).T
    W3p = W3p.astype(bf16)
    b1v = np.asarray(b1, np.float32).astype(bf16)[None, :]
    b2v = np.asarray(b2, np.float32).astype(bf16)[None, :]
    b3v = np.zeros([1, 16], np.float32)
    b3v[0, :C] = np.asarray(b3, np.float32)
    b3v = b3v.astype(bf16)
    Rfl = np.tile(np.asarray(R, np.float32).reshape(1, C * C), (128, 1))
    iota = np.tile(np.arange(128, dtype=np.float32).astype(bf16), (128, 1))
    identf = np.eye(16, dtype=np.float32)

    def pad_ch(v):
        z = np.zeros(CH, v.dtype)
        z[:SH] = v
        return z

    in_maps = []
    for k in range(NCORES):
        sl = slice(k * SH, (k + 1) * SH)
        d2 = np.tile(pad_ch((dinv[sl] ** 2).astype(np.float32)).astype(bf16),
                     (128, 1))
        cs = pad_ch(colsum_full[sl].astype(np.float32)).astype(bf16)[None, :]
        (g_lo, c_lo), (g_hi, c_hi) = gidx[k]
        in_maps.append({
            "T1in": T1, "W1Tp": W1p, "W2T": W2Tb, "W3Tp": W3p,
            "b1r": b1v, "b2r": b2v, "b3r": b3v,
            "dinv2b": d2, "colsum": cs,
            "iota_in": iota, "identf": identf, "Rfl": Rfl,
            "gi_lo": g_lo, "gi_hi": g_hi, "cr_lo": c_lo, "cr_hi": c_hi,
        })

    trace = os.environ.get("GNN_TRACE") == "1"
    res = run_bass_kernel_spmd(nc, in_maps, core_ids=list(range(NCORES)),
                               trace=trace)
    LAST_RESULTS = res

    reps = int(os.environ.get("GNN_BENCH", "0"))
    if reps > 0:
        _bench(nc, in_maps, reps)
    return np.concatenate([res.results[k]["out"] for k in range(NCORES)], axis=0)


BENCH_TIMES = None
BENCH_PIPELINED_NS = None


def _bench(nc, in_maps, reps):
    """Time repeated executions of the already-built program through a single
    jit instance (NEFF compile amortized away; inputs device_put once)."""
    global BENCH_TIMES
    import time
    import jax
    import numpy as jnp_np
    from jax.sharding import Mesh, PartitionSpec, NamedSharding
    from jax.experimental.shard_map import shard_map
    import concourse.mybir as mybir
    from concourse.bass2jax import (_bass_exec_p, partition_id_tensor,
                                    install_neuronx_cc_hook)

    install_neuronx_cc_hook()
    in_names, out_names, out_avals, zero_outs = [], [], [], []
    pname = nc.partition_id_tensor.name if nc.partition_id_tensor else None
    for alloc in nc.m.functions[0].allocations:
        if not isinstance(alloc, mybir.MemoryLocationSet):
            continue
        name = alloc.memorylocations[0].name
        if alloc.kind == "ExternalInput":
            if name != pname:
                in_names.append(name)
        elif alloc.kind == "ExternalOutput":
            out_names.append(name)
            shape = tuple(alloc.tensor_shape)
            dtype = mybir.dt.np(alloc.dtype)
            out_avals.append(jax.core.ShapedArray(shape, dtype))
            zero_outs.append(np.zeros(shape, dtype))
    n_params = len(in_names)
    all_names = in_names + out_names + ([pname] if pname else [])

    def _body(*args):
        ops = list(args)
        if pname:
            ops.append(partition_id_tensor())
        return tuple(_bass_exec_p.bind(
            *ops, out_avals=tuple(out_avals), in_names=tuple(all_names),
            out_names=tuple(out_names), lowering_input_output_aliases=(),
            sim_require_finite=True, sim_require_nnan=True, nc=nc))

    devices = jax.devices()[:NCORES]
    mesh = Mesh(np.asarray(devices), ("core",))
    nouts = len(out_names)
    sharded = jax.jit(
        shard_map(_body, mesh=mesh,
                  in_specs=(PartitionSpec("core"),) * (n_params + nouts),
                  out_specs=(PartitionSpec("core"),) * nouts, check_rep=False),
        donate_argnums=tuple(range(n_params, n_params + nouts)),
        keep_unused=True)
    sh = NamedSharding(mesh, PartitionSpec("core"))
    dev_in = [jax.device_put(
        np.concatenate([np.asarray(in_maps[c][nm]) for c in range(NCORES)], axis=0), sh)
        for nm in in_names]
    times = []
    for i in range(reps + 1):
        zs = [jax.device_put(
            np.zeros((NCORES * z.shape[0], *z.shape[1:]), z.dtype), sh)
            for z in zero_outs]
        t0 = time.perf_counter()
        outs = sharded(*dev_in, *zs)
        jax.block_until_ready(outs)
        times.append(time.perf_counter() - t0)
    BENCH_TIMES = times
    print("bench wall times (s):", " ".join(f"{t:.4f}" for t in times))
    print(f"bench min/median after warmup: {min(times[1:]):.4f} / "
          f"{sorted(times[1:])[len(times[1:]) // 2]:.4f}")

    # pipelined async dispatch: amortizes per-call RPC overhead. Measure two
    # pipeline depths and take the slope to cancel fixed batch overhead.
    def pipe_time(npipe):
        zss = [[jax.device_put(
            np.zeros((NCORES * z.shape[0], *z.shape[1:]), z.dtype), sh)
            for z in zero_outs] for _ in range(npipe)]
        t0 = time.perf_counter()
        outs = None
        for i in range(npipe):
            outs = sharded(*dev_in, *zss[i])
        jax.block_until_ready(outs)
        return time.perf_counter() - t0

    n_lo, n_hi = 16, 96
    # device clocks ramp up only after ~10-15s of sustained load; warm with
    # continuous pipes for a fixed wall budget before measuring.
    warm_s = float(os.environ.get("GNN_WARM_S", "14"))
    t0 = time.perf_counter()
    while time.perf_counter() - t0 < warm_s:
        cur = pipe_time(n_hi) / n_hi
        print(f"warm: {cur * 1e3:.2f} ms/exec")
    slopes, his = [], []
    for _ in range(int(os.environ.get("GNN_SLOPE_REPS", "5"))):
        t_lo = pipe_time(n_lo)
        t_hi = pipe_time(n_hi)
        slope = (t_hi - t_lo) / (n_hi - n_lo)
        print(f"bench pipe: T{n_lo}={t_lo:.4f}s T{n_hi}={t_hi:.4f}s "
              f"slope={slope * 1e3:.3f} ms/exec")
        his.append(t_hi / n_hi)
        if slope > 0:
            slopes.append(slope)
    # noise guard: machine jitter can corrupt individual rounds in either
    # direction (even negative slopes). The median positive slope is robust;
    # cap at the best amortized T_hi/n_hi (an upper bound on per-exec time).
    tp = sorted(slopes)[len(slopes) // 2] if slopes else min(his)
    tp = min(tp, min(his))
    global BENCH_PIPELINED_NS
    BENCH_PIPELINED_NS = int(tp * 1e9)
    print(f"bench pipelined per-exec: {tp * 1e3:.3f} ms "
          f"({tp * 1e9:.0f} ns upper bound)")

